# revision 23
# baseline (speedup 1.0000x reference)
"""Multi-head causal attention (B=4, S=2048, D=1024, H=16) on 8 TRN2 cores.

Sharding: 8 cores = 4 batches x 2 head-groups (tensor-parallel over heads).
Each core computes, for its (batch b, head-group g of 8 heads):
  - Q^T, K^T projections in transposed layout [512 head-dims, 2048 tokens]
  - V projection in natural layout [2048 tokens, 512 dims], padded with a
    ones-column per head (denominator trick)
  - causal attention per head entirely in the transposed domain:
      scoresT[k, q] = K_h Q_h^T  (one matmul per 128k x 512q block,
      lower-triangle blocks only), P^T = exp(scoresT / 8) * causal_mask,
      raw^T[d|sum, q] = [V_h | 1]^T P^T  (PSUM row 64 = softmax denominator)
      attnT = raw^T[0:64] * bcast(1/denom)  (partition-broadcast via a
      DRAM bounce, since DVE cannot broadcast across partitions)
  - output projection partial: outT[e, q] = Wo_g^T attnT  [1024, 2048]
Host sums the two head-group partials per batch, transposes back, adds bo.

Matmul operands are bf16 (hosts casts inputs); accumulation is fp32 in PSUM;
softmax (exp, reciprocal, normalize) is fp32.
"""
import sys

sys.path.insert(0, "/opt/trn_rl_repo")

import ml_dtypes
import numpy as np

import concourse.bass as bass
import concourse.mybir as mybir
import concourse.tile as tile
from concourse.bass_utils import run_bass_kernel_spmd
from concourse.vector_clock import ScopedClock

B, S, D, H = 4, 2048, 1024, 16
DK = 64          # head dim
HG = 8           # heads per core
DG = 512         # dims per core (head-group width)
NQT = 4          # q tiles of 512
NKT = 16         # k tiles of 128
NDI = 8          # contraction chunks of 128 over D
BF16 = mybir.dt.bfloat16
F32 = mybir.dt.float32
EXP = mybir.ActivationFunctionType.Exp
IDENT = mybir.ActivationFunctionType.Identity

# ---------------------------------------------------------------------------
# The AWS walrus CTRL-class codegen accepts only ONE sync-wait per NoOp/Drain,
# but Tile's kernel-tail drain attaches one wait per live semaphore. Spill the
# waits one-per-NOP before the drain.
_PATCHED = False


def _patch_tile_drain():
    global _PATCHED
    if _PATCHED:
        return
    _PATCHED = True

    def patched_drain_and_barrier(self, tick_clock, wait_clock):
        nop_inst = self.nc.sync.nop(nofuse=True)
        wait_clock.add_sem_waits(
            nop_inst.ins, ScopedClock({None: tick_clock.global_clock})
        )
        si = nop_inst.ins.sync_info
        waits = list(si.on_wait or []) if si is not None else []
        if len(waits) > 1:
            si.on_wait = waits[:1]
            rest = waits[1:]
            while rest:
                extra = self.nc.sync.nop(nofuse=True)
                extra.ins.sync_info = mybir.SyncInfo(
                    on_wait=rest[:1], on_update=[]
                )
                rest = rest[1:]
        self.nc.sync.drain()
        self.nc.all_engine_barrier()
        assert self.sems is not None
        popped = self.nc._tile_sem_poison_stack.pop()
        assert popped is self._sem_poison
        self.nc.clear_and_free_semaphores(list(self.sems.allocated().values()))
        self.nc.all_engine_barrier()

    tile.TileContext._drain_and_barrier = patched_drain_and_barrier


def _split_multi_waits(nc):
    """The AWS walrus codegen accepts at most ONE sync-wait command per
    instruction (any class). Tile attaches one wait per producer proc. Hoist
    the extra waits onto nofuse NOPs inserted just before the instruction on
    the same engine — per-engine streams execute in order, so blocking
    semantics are preserved."""
    for bb in nc.main_func.blocks:
        new_insts = []
        for ins in bb.instructions:
            si = ins.sync_info
            if si is not None and si.on_wait and len(si.on_wait) > 1:
                waits = list(si.on_wait)
                si.on_wait = waits[-1:]
                for k, w in enumerate(waits[:-1]):
                    nop = mybir.InstNoOp(
                        name=f"{ins.name}_hw{k}",
                        sync_info=mybir.SyncInfo(on_wait=[w], on_update=[]),
                        bass_nofuse=True,
                        engine=ins.engine,
                    )
                    new_insts.append(nop)
            new_insts.append(ins)
        bb.instructions[:] = new_insts


# ---------------------------------------------------------------------------
def build_program():
    _patch_tile_drain()
    nc = bass.Bass()

    xqT = nc.dram_tensor("xqT", [D, S], BF16, kind="ExternalInput")
    xkT = nc.dram_tensor("xkT", [D, S], BF16, kind="ExternalInput")
    xvT = nc.dram_tensor("xvT", [D, S], BF16, kind="ExternalInput")
    wqT = nc.dram_tensor("wqT", [D, DG], BF16, kind="ExternalInput")
    wkT = nc.dram_tensor("wkT", [D, DG], BF16, kind="ExternalInput")
    wvT = nc.dram_tensor("wvT", [D, DG], BF16, kind="ExternalInput")
    woT = nc.dram_tensor("woT", [DG, D], BF16, kind="ExternalInput")
    bqd = nc.dram_tensor("bq", [128, 4], F32, kind="ExternalInput")
    bkd = nc.dram_tensor("bk", [128, 4], F32, kind="ExternalInput")
    maskd = nc.dram_tensor("maskT", [128, 2048], BF16, kind="ExternalInput")
    outT = nc.dram_tensor("outT", [D, S], F32, kind="ExternalOutput")

    with tile.TileContext(nc) as tc:
        with (
            tc.tile_pool(name="const", bufs=1) as const,
            tc.tile_pool(name="persist", bufs=1) as persist,
            tc.tile_pool(name="xq", bufs=1) as xq_pool,
            tc.tile_pool(name="xk", bufs=1) as xk_pool,
            tc.tile_pool(name="pt", bufs=4) as pt_pool,
            tc.tile_pool(name="raw", bufs=4) as raw_pool,
            tc.tile_pool(name="rc", bufs=3) as rc_pool,
            tc.tile_pool(name="rb", bufs=4) as rb_pool,
            tc.tile_pool(name="outp", bufs=4) as out_pool,
            tc.tile_pool(name="mm_ps", bufs=2, space="PSUM") as mm_psum,
            tc.tile_pool(name="sc_ps", bufs=2, space="PSUM") as sc_psum,
            tc.tile_pool(name="pv_ps", bufs=2, space="PSUM") as pv_psum,
            tc.tile_pool(name="dram", bufs=8, space="DRAM") as dram_pool,
        ):
            # ---- constants / weights / inputs ----------------------------
            # Load order matters: the first Q-projection group needs wq and
            # xq chunk-by-chunk, so those DMAs go first (interleaved); the
            # K/V/O weights and the mask are only needed later.
            bq_sb = const.tile([128, 4], F32, tag="bq", name="bq_sb")
            bk_sb = const.tile([128, 4], F32, tag="bk", name="bk_sb")
            nc.sync.dma_start(bq_sb[:], bqd[:])
            nc.sync.dma_start(bk_sb[:], bkd[:])
            WQ, XQ = [], []
            for di in range(NDI):
                w = const.tile([128, DG], BF16, tag=f"wq{di}", name=f"wq{di}")
                nc.sync.dma_start(w[:], wqT[di * 128:(di + 1) * 128, :])
                WQ.append(w)
                x = xq_pool.tile([128, S], BF16, tag=f"xq{di}", name=f"xq{di}")
                nc.sync.dma_start(x[:], xqT[di * 128:(di + 1) * 128, :])
                XQ.append(x)
            WK, XK = [], []
            for di in range(NDI):
                w = const.tile([128, DG], BF16, tag=f"wk{di}", name=f"wk{di}")
                nc.sync.dma_start(w[:], wkT[di * 128:(di + 1) * 128, :])
                WK.append(w)
                x = xk_pool.tile([128, S], BF16, tag=f"xk{di}", name=f"xk{di}")
                nc.sync.dma_start(x[:], xkT[di * 128:(di + 1) * 128, :])
                XK.append(x)
            WV = []
            for di in range(NDI):
                w = const.tile([128, DG], BF16, tag=f"wv{di}", name=f"wv{di}")
                nc.sync.dma_start(w[:], wvT[di * 128:(di + 1) * 128, :])
                WV.append(w)
            WO = []
            for c in range(4):
                w = const.tile([128, D], BF16, tag=f"wo{c}", name=f"wo{c}")
                nc.sync.dma_start(w[:], woT[c * 128:(c + 1) * 128, :])
                WO.append(w)
            maskT = const.tile([128, 2048], BF16, tag="mask", name="mask")
            nc.sync.dma_start(maskT[:], maskd[:])

            # ---- persistent intermediates --------------------------------
            QT = [persist.tile([128, S], BF16, tag=f"qt{t}", name=f"qt{t}") for t in range(4)]
            KT = [persist.tile([128, S], BF16, tag=f"kt{t}", name=f"kt{t}") for t in range(4)]
            AT = [persist.tile([128, S], BF16, tag=f"at{t}", name=f"at{t}") for t in range(4)]
            VP = [persist.tile([128, HG * 65], BF16, tag=f"vp{t}", name=f"vp{t}")
                  for t in range(NKT)]

            # ---- projection building blocks ------------------------------
            def proj_block(Xs, Ws, OutT, bias_sb, st, dot):
                ps = mm_psum.tile([128, 512], F32, tag="mm", name="mm_ps")
                for di in range(NDI):
                    nc.tensor.matmul(
                        ps[:],
                        Ws[di][:, dot * 128:(dot + 1) * 128],
                        Xs[di][:, st * 512:(st + 1) * 512],
                        start=(di == 0),
                        stop=(di == NDI - 1),
                    )
                nc.vector.tensor_scalar_add(
                    OutT[dot][:, st * 512:(st + 1) * 512],
                    ps[:],
                    bias_sb[:, dot:dot + 1],
                )

            def vproj_kt(XV, kt):
                ps = mm_psum.tile([128, 512], F32, tag="mm", name="mm_ps")
                for di in range(NDI):
                    nc.tensor.matmul(
                        ps[:],
                        XV[di][:, kt * 128:(kt + 1) * 128],
                        WV[di][:],
                        start=(di == 0),
                        stop=(di == NDI - 1),
                    )
                # scatter the 8x64 head slices into the 8x65 padded layout
                nc.vector.tensor_copy(
                    VP[kt][:].rearrange("p (h c) -> p h c", h=HG)[:, :, 0:64],
                    ps[:].rearrange("p (h c) -> p h c", h=HG),
                )
                nc.vector.memset(
                    VP[kt][:].rearrange("p (h c) -> p h c", h=HG)[:, :, 64:65],
                    1.0,
                )

            # Q projection in full (QT is read by every attention phase),
            # then only what attention j=0 needs (KT s-tile 0, VP k-tiles
            # 0-3). The rest of the K/V projections interleave between
            # attention-j=0 heads as PE filler, so PE stays dense while the
            # ACT exp stream paces the attention chains.
            for st in range(NQT):
                for dot in range(4):
                    proj_block(XQ, WQ, QT, bq_sb, st, dot)
            XV = []
            for di in range(NDI):
                x = xq_pool.tile([128, S], BF16, tag=f"xq{di}", name=f"xq{di}")
                nc.sync.dma_start(x[:], xvT[di * 128:(di + 1) * 128, :])
                XV.append(x)
            for dot in range(4):
                proj_block(XK, WK, KT, bk_sb, 0, dot)
            for kt in range(4):
                vproj_kt(XV, kt)

            def kblock(st, dot):
                return lambda: proj_block(XK, WK, KT, bk_sb, st, dot)

            def vblock(kt):
                return lambda: vproj_kt(XV, kt)

            # ---- attention + output projection per q-tile ----------------
            # Per (head, q-tile): k-chunk PAIRS. Each pair does two score
            # matmuls into a 2-bank PSUM tile, ONE exp over [128, 1024], one
            # mask-mul for diagonal pairs, two PV matmuls. The PV pair lags
            # the score pair by PV_LAG so PE never waits on the ACT exp.
            # Output projection for q-tile j-1 interleaves between heads of
            # phase j to keep dense PE work over the attnT normalize chains.
            PV_LAG = 2

            def outproj_block(j, et):
                op = mm_psum.tile([128, 512], F32, tag="mm", name="mm_ps")
                for c in range(4):
                    nc.tensor.matmul(
                        op[:],
                        WO[c][:, et * 128:(et + 1) * 128],
                        AT[c][:, j * 512:(j + 1) * 512],
                        start=(c == 0),
                        stop=(c == 3),
                    )
                ot = out_pool.tile([128, 512], F32, tag="ot", name="ot")
                nc.vector.tensor_copy(ot[:], op[:])
                nc.sync.dma_start(
                    outT[et * 128:(et + 1) * 128, j * 512:(j + 1) * 512],
                    ot[:],
                )

            def attention_head(h, j, fill_queue=None, max_pops=0):
                nk = 4 * j + 4          # causal: k-chunks 0..nk-1
                npairs = nk // 2
                rt, r = h // 2, 64 * (h % 2)
                qs = slice(j * 512, (j + 1) * 512)
                pv = pv_psum.tile([65, 512], F32, tag="pv", name="pv_ps")
                pts = {}
                pops = 0

                def emit_pv(p):
                    pt = pts.pop(p)
                    for half in range(2):
                        i = 2 * p + half
                        nc.tensor.matmul(
                            pv[:],
                            VP[i][:, 65 * h:65 * h + 65],
                            pt[:, half * 512:(half + 1) * 512],
                            start=(i == 0),
                            stop=(i == nk - 1),
                        )

                for p in range(npairs):
                    sc = sc_psum.tile([128, 1024], F32, tag="sc", name="sc_ps")
                    for half in range(2):
                        i = 2 * p + half
                        nc.tensor.matmul(
                            sc[:, half * 512:(half + 1) * 512],
                            KT[rt][r:r + 64, i * 128:(i + 1) * 128],
                            QT[rt][r:r + 64, qs],
                            start=True,
                            stop=True,
                        )
                    pt = pt_pool.tile([128, 1024], BF16, tag="pt", name="pt")
                    if p == 2 * j + 1:
                        # last diagonal pair (offsets 256/384): cols 0-255
                        # are fully masked; skip their exp, zero them, and
                        # mask the rest on DVE (latency-critical tail)
                        nc.scalar.activation(
                            pt[:, 256:1024], sc[:, 256:1024], EXP, scale=0.125
                        )
                        nc.vector.memset(pt[:, 0:256], 0.0)
                        nc.vector.tensor_mul(
                            pt[:, 256:1024], pt[:, 256:1024],
                            maskT[:, 1280:2048],
                        )
                    elif p == 2 * j:
                        nc.scalar.activation(pt[:], sc[:], EXP, scale=0.125)
                        nc.vector.tensor_mul(pt[:], pt[:], maskT[:, 0:1024])
                    else:
                        nc.scalar.activation(pt[:], sc[:], EXP, scale=0.125)
                    pts[p] = pt
                    # PE filler between pairs: keeps PE dense while ACT's
                    # exp stream catches up (ACT paces late phases otherwise)
                    if (fill_queue and pops < max_pops and p % 2 == 0):
                        fill_queue.pop(0)()
                        pops += 1
                    if p >= PV_LAG:
                        emit_pv(p - PV_LAG)
                for p in range(max(0, npairs - PV_LAG), npairs):
                    emit_pv(p)

                # Evict PV to SBUF right away (frees the PSUM bank so the
                # next heads' PV groups never stall PE). The normalize chain
                # runs off the critical path: bounce the denominator row
                # through DRAM into a [128, 4] partition-spread layout so
                # DVE reciprocal runs 128 lanes wide (a [1, 512] reciprocal
                # is one lane and costs 3.3us, stalling the DVE queue), then
                # bounce the reciprocals back out as a [64, 512] partition
                # broadcast for the normalize multiply.
                raw = raw_pool.tile([65, 512], F32, tag="raw", name="raw")
                nc.vector.tensor_copy(raw[:], pv[:])
                dta = dram_pool.tile([1, 512], F32, tag="dna", name="dna")
                nc.sync.dma_start(dta[:], raw[64:65, :])
                den4 = rc_pool.tile([128, 4], F32, tag="den4", name="den4")
                nc.sync.dma_start(
                    den4[:], dta[:].rearrange("a (p c) -> (a p) c", c=4)
                )
                rcp4 = rc_pool.tile([128, 4], F32, tag="rcp4", name="rcp4")
                nc.vector.reciprocal(rcp4[:], den4[:])
                dtb = dram_pool.tile([128, 4], F32, tag="dnb", name="dnb")
                nc.sync.dma_start(dtb[:], rcp4[:])
                rb = rb_pool.tile([64, 512], F32, tag="rb", name="rb")
                nc.sync.dma_start(
                    rb[:],
                    dtb[:].rearrange("p c -> (p c)")[None, :]
                    .to_broadcast((64, 512)),
                )
                nc.vector.tensor_mul(AT[rt][r:r + 64, qs], raw[0:64, :], rb[:])

            # Filler distribution keeps every phase PE-rich vs its ACT load:
            # phase j consumes the K-projection s-tile j+1 and V-projection
            # k-tiles for phase j+1; all deferred output projections land in
            # phase 3 (the most exp-heavy), 3 blocks per head.
            queues = {
                0: [kblock(1, d) for d in range(4)] + [vblock(t) for t in range(4, 8)],
                1: [kblock(2, d) for d in range(4)] + [vblock(t) for t in range(8, 12)],
                2: [kblock(3, d) for d in range(4)] + [vblock(t) for t in range(12, 16)],
                3: [lambda jj=jj, et=et: outproj_block(jj, et)
                    for jj in range(3) for et in range(8)],
            }
            pops = {0: 1, 1: 1, 2: 1, 3: 3}
            for j in range(NQT):
                for h in range(HG):
                    attention_head(h, j, queues[j], pops[j])
                # drain any leftovers before the next phase needs them
                while queues[j]:
                    queues[j].pop(0)()
            # Final output projection: process et-blocks in pairs with the
            # head-chunk (c) loop outermost, so the c<3 matmuls (which only
            # need the earlier heads' attnT) run while the last heads'
            # normalize chains are still in flight.
            for pair in range(4):
                ops = []
                for k in range(2):
                    ops.append(
                        mm_psum.tile([128, 512], F32, tag="mm", name="mm_ps"))
                for c in range(4):
                    for k in range(2):
                        et = 2 * pair + k
                        nc.tensor.matmul(
                            ops[k][:],
                            WO[c][:, et * 128:(et + 1) * 128],
                            AT[c][:, (NQT - 1) * 512:NQT * 512],
                            start=(c == 0),
                            stop=(c == 3),
                        )
                for k in range(2):
                    et = 2 * pair + k
                    ot = out_pool.tile([128, 512], F32, tag="ot", name="ot")
                    nc.vector.tensor_copy(ot[:], ops[k][:])
                    nc.sync.dma_start(
                        outT[et * 128:(et + 1) * 128,
                             (NQT - 1) * 512:NQT * 512],
                        ot[:],
                    )
    _split_multi_waits(nc)
    return nc


_PROGRAM = None


def _get_program():
    global _PROGRAM
    if _PROGRAM is None:
        _PROGRAM = build_program()
    return _PROGRAM


def _make_in_maps(query, key, value, Wq, bq, Wk, bk, Wv, bv, Wo):
    bf = ml_dtypes.bfloat16
    # pair-masks for the two diagonal k-chunk pairs of each q-tile:
    # block offsets (0, 128) and (256, 384); keep iff q >= k + o
    k_idx = np.arange(128, dtype=np.int32)[:, None]
    q_idx = np.arange(512, dtype=np.int32)[None, :]
    halves = [(q_idx >= k_idx + o) for o in (0, 128, 256, 384)]
    maskT = np.concatenate(halves, axis=1).astype(bf)
    in_maps = []
    for c in range(8):
        b, g = divmod(c, 2)
        gs = slice(DG * g, DG * (g + 1))
        in_maps.append({
            "xqT": np.asarray(query[b], np.float32).T.astype(bf, order="C"),
            "xkT": np.asarray(key[b], np.float32).T.astype(bf, order="C"),
            "xvT": np.asarray(value[b], np.float32).T.astype(bf, order="C"),
            "wqT": np.asarray(Wq[gs, :], np.float32).T.astype(bf, order="C"),
            "wkT": np.asarray(Wk[gs, :], np.float32).T.astype(bf, order="C"),
            "wvT": np.asarray(Wv[gs, :], np.float32).T.astype(bf, order="C"),
            "woT": np.asarray(Wo[:, gs], np.float32).T.astype(bf, order="C"),
            "bq": np.ascontiguousarray(
                np.asarray(bq[gs], np.float32).reshape(4, 128).T),
            "bk": np.ascontiguousarray(
                np.asarray(bk[gs], np.float32).reshape(4, 128).T),
            "maskT": maskT,
        })
    return in_maps


def run(query, key, value, Wq, bq, Wk, bk, Wv, bv, Wo, bo, trace=False,
        **spmd_kwargs):
    nc = _get_program()
    in_maps = _make_in_maps(query, key, value, Wq, bq, Wk, bk, Wv, bv, Wo)
    res = run_bass_kernel_spmd(nc, in_maps, list(range(8)), trace=trace,
                               **spmd_kwargs)
    out = np.empty((B, S, D), np.float32)
    for b in range(B):
        out[b] = (res.results[2 * b]["outT"] + res.results[2 * b + 1]["outT"]).T
    # bv is folded in host-side: attn rows sum to 1 after softmax, so
    # out += Wo @ bv exactly accounts for the V bias.
    bias = np.asarray(bo, np.float32) + \
        np.asarray(Wo, np.float32) @ np.asarray(bv, np.float32)
    out += bias[None, None, :]
    return out, res


def kernel(query, key, value, mask, Wq, bq, Wk, bk, Wv, bv, Wo, bo):
    out, _ = run(query, key, value, Wq, bq, Wk, bk, Wv, bv, Wo, bo)
    return out


# revision 24
# speedup vs baseline: 1.1063x; 1.1063x over previous
"""Multi-head causal attention (B=4, S=2048, D=1024, H=16) on 8 TRN2 cores.

Sharding: 8 cores = 4 batches x 2 head-groups (tensor-parallel over heads).
Each core computes, for its (batch b, head-group g of 8 heads):
  - Q^T, K^T projections in transposed layout [512 head-dims, 2048 tokens]
  - V projection in natural layout [2048 tokens, 512 dims], padded with a
    ones-column per head (denominator trick)
  - causal attention per head entirely in the transposed domain:
      scoresT[k, q] = K_h Q_h^T  (one matmul per 128k x 512q block,
      lower-triangle blocks only), P^T = exp(scoresT / 8) * causal_mask,
      raw^T[d|sum, q] = [V_h | 1]^T P^T  (PSUM row 64 = softmax denominator)
      attnT = raw^T[0:64] * bcast(1/denom)  (partition-broadcast via a
      DRAM bounce, since DVE cannot broadcast across partitions)
  - output projection partial: outT[e, q] = Wo_g^T attnT  [1024, 2048]
Host sums the two head-group partials per batch, transposes back, adds bo.

Matmul operands are bf16 (hosts casts inputs); accumulation is fp32 in PSUM;
softmax (exp, reciprocal, normalize) is fp32.
"""
import sys

sys.path.insert(0, "/opt/trn_rl_repo")

import ml_dtypes
import numpy as np

import concourse.bass as bass
import concourse.mybir as mybir
import concourse.tile as tile
from concourse.bass_utils import run_bass_kernel_spmd
from concourse.vector_clock import ScopedClock

B, S, D, H = 4, 2048, 1024, 16
DK = 64          # head dim
HG = 8           # heads per core
DG = 512         # dims per core (head-group width)
NQT = 4          # q tiles of 512
NKT = 16         # k tiles of 128
NDI = 8          # contraction chunks of 128 over D
BF16 = mybir.dt.bfloat16
F32 = mybir.dt.float32
EXP = mybir.ActivationFunctionType.Exp
IDENT = mybir.ActivationFunctionType.Identity

# ---------------------------------------------------------------------------
# The AWS walrus CTRL-class codegen accepts only ONE sync-wait per NoOp/Drain,
# but Tile's kernel-tail drain attaches one wait per live semaphore. Spill the
# waits one-per-NOP before the drain.
_PATCHED = False


def _patch_tile_drain():
    global _PATCHED
    if _PATCHED:
        return
    _PATCHED = True

    def patched_drain_and_barrier(self, tick_clock, wait_clock):
        nop_inst = self.nc.sync.nop(nofuse=True)
        wait_clock.add_sem_waits(
            nop_inst.ins, ScopedClock({None: tick_clock.global_clock})
        )
        si = nop_inst.ins.sync_info
        waits = list(si.on_wait or []) if si is not None else []
        if len(waits) > 1:
            si.on_wait = waits[:1]
            rest = waits[1:]
            while rest:
                extra = self.nc.sync.nop(nofuse=True)
                extra.ins.sync_info = mybir.SyncInfo(
                    on_wait=rest[:1], on_update=[]
                )
                rest = rest[1:]
        self.nc.sync.drain()
        self.nc.all_engine_barrier()
        assert self.sems is not None
        popped = self.nc._tile_sem_poison_stack.pop()
        assert popped is self._sem_poison
        self.nc.clear_and_free_semaphores(list(self.sems.allocated().values()))
        self.nc.all_engine_barrier()

    tile.TileContext._drain_and_barrier = patched_drain_and_barrier


def _split_multi_waits(nc):
    """The AWS walrus codegen accepts at most ONE sync-wait command per
    instruction (any class). Tile attaches one wait per producer proc. Hoist
    the extra waits onto nofuse NOPs inserted just before the instruction on
    the same engine — per-engine streams execute in order, so blocking
    semantics are preserved."""
    for bb in nc.main_func.blocks:
        new_insts = []
        for ins in bb.instructions:
            si = ins.sync_info
            if si is not None and si.on_wait and len(si.on_wait) > 1:
                waits = list(si.on_wait)
                si.on_wait = waits[-1:]
                for k, w in enumerate(waits[:-1]):
                    nop = mybir.InstNoOp(
                        name=f"{ins.name}_hw{k}",
                        sync_info=mybir.SyncInfo(on_wait=[w], on_update=[]),
                        bass_nofuse=True,
                        engine=ins.engine,
                    )
                    new_insts.append(nop)
            new_insts.append(ins)
        bb.instructions[:] = new_insts


# ---------------------------------------------------------------------------
def build_program():
    _patch_tile_drain()
    nc = bass.Bass()

    xqT = nc.dram_tensor("xqT", [D, S], BF16, kind="ExternalInput")
    xkT = nc.dram_tensor("xkT", [D, S], BF16, kind="ExternalInput")
    xvT = nc.dram_tensor("xvT", [D, S], BF16, kind="ExternalInput")
    wqT = nc.dram_tensor("wqT", [D, DG], BF16, kind="ExternalInput")
    wkT = nc.dram_tensor("wkT", [D, DG], BF16, kind="ExternalInput")
    wvT = nc.dram_tensor("wvT", [D, DG], BF16, kind="ExternalInput")
    woT = nc.dram_tensor("woT", [DG, D], BF16, kind="ExternalInput")
    bqd = nc.dram_tensor("bq", [128, 4], F32, kind="ExternalInput")
    bkd = nc.dram_tensor("bk", [128, 4], F32, kind="ExternalInput")
    maskd = nc.dram_tensor("maskT", [128, 2048], BF16, kind="ExternalInput")
    outT = nc.dram_tensor("outT", [D, S], F32, kind="ExternalOutput")

    with tile.TileContext(nc) as tc:
        with (
            tc.tile_pool(name="const", bufs=1) as const,
            tc.tile_pool(name="persist", bufs=1) as persist,
            tc.tile_pool(name="xq", bufs=1) as xq_pool,
            tc.tile_pool(name="xk", bufs=1) as xk_pool,
            tc.tile_pool(name="pt", bufs=4) as pt_pool,
            tc.tile_pool(name="raw", bufs=4) as raw_pool,
            tc.tile_pool(name="rc", bufs=3) as rc_pool,
            tc.tile_pool(name="rb", bufs=4) as rb_pool,
            tc.tile_pool(name="outp", bufs=4) as out_pool,
            tc.tile_pool(name="mm_ps", bufs=2, space="PSUM") as mm_psum,
            tc.tile_pool(name="sc_ps", bufs=2, space="PSUM") as sc_psum,
            tc.tile_pool(name="pv_ps", bufs=2, space="PSUM") as pv_psum,
            tc.tile_pool(name="dram", bufs=8, space="DRAM") as dram_pool,
        ):
            # ---- constants / weights / inputs ----------------------------
            # Load order matters: the first Q-projection group needs wq and
            # xq chunk-by-chunk, so those DMAs go first (interleaved); the
            # K/V/O weights and the mask are only needed later.
            bq_sb = const.tile([128, 4], F32, tag="bq", name="bq_sb")
            bk_sb = const.tile([128, 4], F32, tag="bk", name="bk_sb")
            nc.sync.dma_start(bq_sb[:], bqd[:])
            nc.sync.dma_start(bk_sb[:], bkd[:])
            WQ, XQ = [], []
            for di in range(NDI):
                w = const.tile([128, DG], BF16, tag=f"wq{di}", name=f"wq{di}")
                nc.sync.dma_start(w[:], wqT[di * 128:(di + 1) * 128, :])
                WQ.append(w)
                x = xq_pool.tile([128, S], BF16, tag=f"xq{di}", name=f"xq{di}")
                nc.sync.dma_start(x[:], xqT[di * 128:(di + 1) * 128, :])
                XQ.append(x)
            WK, XK = [], []
            for di in range(NDI):
                w = const.tile([128, DG], BF16, tag=f"wk{di}", name=f"wk{di}")
                nc.sync.dma_start(w[:], wkT[di * 128:(di + 1) * 128, :])
                WK.append(w)
                x = xk_pool.tile([128, S], BF16, tag=f"xk{di}", name=f"xk{di}")
                nc.sync.dma_start(x[:], xkT[di * 128:(di + 1) * 128, :])
                XK.append(x)
            WV = []
            for di in range(NDI):
                w = const.tile([128, DG], BF16, tag=f"wv{di}", name=f"wv{di}")
                nc.sync.dma_start(w[:], wvT[di * 128:(di + 1) * 128, :])
                WV.append(w)
            WO = []
            for c in range(4):
                w = const.tile([128, D], BF16, tag=f"wo{c}", name=f"wo{c}")
                nc.sync.dma_start(w[:], woT[c * 128:(c + 1) * 128, :])
                WO.append(w)
            maskT = const.tile([128, 2048], BF16, tag="mask", name="mask")
            nc.sync.dma_start(maskT[:], maskd[:])

            # ---- persistent intermediates --------------------------------
            QT = [persist.tile([128, S], BF16, tag=f"qt{t}", name=f"qt{t}") for t in range(4)]
            KT = [persist.tile([128, S], BF16, tag=f"kt{t}", name=f"kt{t}") for t in range(4)]
            AT = [persist.tile([128, S], BF16, tag=f"at{t}", name=f"at{t}") for t in range(4)]
            VP = [persist.tile([128, HG * 65], BF16, tag=f"vp{t}", name=f"vp{t}")
                  for t in range(NKT)]

            # ---- projection building blocks ------------------------------
            def proj_block(Xs, Ws, OutT, bias_sb, st, dot):
                ps = mm_psum.tile([128, 512], F32, tag="mm", name="mm_ps")
                for di in range(NDI):
                    nc.tensor.matmul(
                        ps[:],
                        Ws[di][:, dot * 128:(dot + 1) * 128],
                        Xs[di][:, st * 512:(st + 1) * 512],
                        start=(di == 0),
                        stop=(di == NDI - 1),
                    )
                nc.vector.tensor_scalar_add(
                    OutT[dot][:, st * 512:(st + 1) * 512],
                    ps[:],
                    bias_sb[:, dot:dot + 1],
                )

            def vproj_kt(XV, kt):
                ps = mm_psum.tile([128, 512], F32, tag="mm", name="mm_ps")
                for di in range(NDI):
                    nc.tensor.matmul(
                        ps[:],
                        XV[di][:, kt * 128:(kt + 1) * 128],
                        WV[di][:],
                        start=(di == 0),
                        stop=(di == NDI - 1),
                    )
                # scatter the 8x64 head slices into the 8x65 padded layout
                nc.vector.tensor_copy(
                    VP[kt][:].rearrange("p (h c) -> p h c", h=HG)[:, :, 0:64],
                    ps[:].rearrange("p (h c) -> p h c", h=HG),
                )
                nc.vector.memset(
                    VP[kt][:].rearrange("p (h c) -> p h c", h=HG)[:, :, 64:65],
                    1.0,
                )

            # Q projection in full (QT is read by every attention phase),
            # then only what attention j=0 needs (KT s-tile 0, VP k-tiles
            # 0-3). The rest of the K/V projections interleave between
            # attention-j=0 heads as PE filler, so PE stays dense while the
            # ACT exp stream paces the attention chains.
            for st in range(NQT):
                for dot in range(4):
                    proj_block(XQ, WQ, QT, bq_sb, st, dot)
            XV = []
            for di in range(NDI):
                x = xq_pool.tile([128, S], BF16, tag=f"xq{di}", name=f"xq{di}")
                nc.sync.dma_start(x[:], xvT[di * 128:(di + 1) * 128, :])
                XV.append(x)
            for dot in range(4):
                proj_block(XK, WK, KT, bk_sb, 0, dot)
            for kt in range(4):
                vproj_kt(XV, kt)

            def kblock(st, dot):
                return lambda: proj_block(XK, WK, KT, bk_sb, st, dot)

            def vblock(kt):
                return lambda: vproj_kt(XV, kt)

            # ---- attention + output projection per q-tile ----------------
            # Per (head, q-tile): k-chunk PAIRS. Each pair does two score
            # matmuls into a 2-bank PSUM tile, ONE exp over [128, 1024], one
            # mask-mul for diagonal pairs, two PV matmuls. The PV pair lags
            # the score pair by PV_LAG so PE never waits on the ACT exp.
            # Output projection for q-tile j-1 interleaves between heads of
            # phase j to keep dense PE work over the attnT normalize chains.
            PV_LAG = 2

            def outproj_block(j, et):
                op = mm_psum.tile([128, 512], F32, tag="mm", name="mm_ps")
                for c in range(4):
                    nc.tensor.matmul(
                        op[:],
                        WO[c][:, et * 128:(et + 1) * 128],
                        AT[c][:, j * 512:(j + 1) * 512],
                        start=(c == 0),
                        stop=(c == 3),
                    )
                ot = out_pool.tile([128, 512], F32, tag="ot", name="ot")
                nc.vector.tensor_copy(ot[:], op[:])
                nc.sync.dma_start(
                    outT[et * 128:(et + 1) * 128, j * 512:(j + 1) * 512],
                    ot[:],
                )

            def attention_head(h, j, fill_queue=None, max_pops=0):
                nk = 4 * j + 4          # causal: k-chunks 0..nk-1
                npairs = nk // 2
                rt, r = h // 2, 64 * (h % 2)
                qs = slice(j * 512, (j + 1) * 512)
                pv = pv_psum.tile([65, 512], F32, tag="pv", name="pv_ps")
                pts = {}
                pops = 0

                def emit_pv(p):
                    pt = pts.pop(p)
                    for half in range(2):
                        i = 2 * p + half
                        nc.tensor.matmul(
                            pv[:],
                            VP[i][:, 65 * h:65 * h + 65],
                            pt[:, half * 512:(half + 1) * 512],
                            start=(i == 0),
                            stop=(i == nk - 1),
                        )

                for p in range(npairs):
                    sc = sc_psum.tile([128, 1024], F32, tag="sc", name="sc_ps")
                    for half in range(2):
                        i = 2 * p + half
                        nc.tensor.matmul(
                            sc[:, half * 512:(half + 1) * 512],
                            KT[rt][r:r + 64, i * 128:(i + 1) * 128],
                            QT[rt][r:r + 64, qs],
                            start=True,
                            stop=True,
                        )
                    pt = pt_pool.tile([128, 1024], BF16, tag="pt", name="pt")
                    nc.scalar.activation(pt[:], sc[:], EXP, scale=0.125)
                    if p >= 2 * j:  # diagonal pair: apply causal mask
                        pi = p - 2 * j  # 0 -> offsets (0,128), 1 -> (256,384)
                        nc.vector.tensor_mul(
                            pt[:], pt[:], maskT[:, pi * 1024:(pi + 1) * 1024]
                        )
                    pts[p] = pt
                    # PE filler between pairs: keeps PE dense while ACT's
                    # exp stream catches up (ACT paces late phases otherwise)
                    if (fill_queue and pops < max_pops and p % 2 == 0):
                        fill_queue.pop(0)()
                        pops += 1
                    if p >= PV_LAG:
                        emit_pv(p - PV_LAG)
                for p in range(max(0, npairs - PV_LAG), npairs):
                    emit_pv(p)

                # Evict PV to SBUF right away (frees the PSUM bank so the
                # next heads' PV groups never stall PE). The normalize chain
                # runs off the critical path: bounce the denominator row
                # through DRAM into a [128, 4] partition-spread layout so
                # DVE reciprocal runs 128 lanes wide (a [1, 512] reciprocal
                # is one lane and costs 3.3us, stalling the DVE queue), then
                # bounce the reciprocals back out as a [64, 512] partition
                # broadcast for the normalize multiply.
                raw = raw_pool.tile([65, 512], F32, tag="raw", name="raw")
                nc.vector.tensor_copy(raw[:], pv[:])
                dta = dram_pool.tile([1, 512], F32, tag="dna", name="dna")
                nc.sync.dma_start(dta[:], raw[64:65, :])
                den4 = rc_pool.tile([128, 4], F32, tag="den4", name="den4")
                nc.sync.dma_start(
                    den4[:], dta[:].rearrange("a (p c) -> (a p) c", c=4)
                )
                rcp4 = rc_pool.tile([128, 4], F32, tag="rcp4", name="rcp4")
                nc.vector.reciprocal(rcp4[:], den4[:])
                dtb = dram_pool.tile([128, 4], F32, tag="dnb", name="dnb")
                nc.sync.dma_start(dtb[:], rcp4[:])
                rb = rb_pool.tile([64, 512], F32, tag="rb", name="rb")
                nc.sync.dma_start(
                    rb[:],
                    dtb[:].rearrange("p c -> (p c)")[None, :]
                    .to_broadcast((64, 512)),
                )
                nc.vector.tensor_mul(AT[rt][r:r + 64, qs], raw[0:64, :], rb[:])

            # Filler distribution keeps every phase PE-rich vs its ACT load:
            # phase j consumes the K-projection s-tile j+1 and V-projection
            # k-tiles for phase j+1; all deferred output projections land in
            # phase 3 (the most exp-heavy), 3 blocks per head.
            queues = {
                0: [kblock(1, d) for d in range(4)] + [vblock(t) for t in range(4, 8)],
                1: [kblock(2, d) for d in range(4)] + [vblock(t) for t in range(8, 12)],
                2: [kblock(3, d) for d in range(4)] + [vblock(t) for t in range(12, 16)],
                3: [lambda jj=jj, et=et: outproj_block(jj, et)
                    for jj in range(3) for et in range(8)],
            }
            pops = {0: 1, 1: 1, 2: 1, 3: 3}
            for j in range(NQT):
                for h in range(HG):
                    attention_head(h, j, queues[j], pops[j])
                # drain any leftovers before the next phase needs them
                while queues[j]:
                    queues[j].pop(0)()
            # Final output projection: process et-blocks in pairs with the
            # head-chunk (c) loop outermost, so the c<3 matmuls (which only
            # need the earlier heads' attnT) run while the last heads'
            # normalize chains are still in flight.
            for pair in range(4):
                ops = []
                for k in range(2):
                    ops.append(
                        mm_psum.tile([128, 512], F32, tag="mm", name="mm_ps"))
                for c in range(4):
                    for k in range(2):
                        et = 2 * pair + k
                        nc.tensor.matmul(
                            ops[k][:],
                            WO[c][:, et * 128:(et + 1) * 128],
                            AT[c][:, (NQT - 1) * 512:NQT * 512],
                            start=(c == 0),
                            stop=(c == 3),
                        )
                for k in range(2):
                    et = 2 * pair + k
                    ot = out_pool.tile([128, 512], F32, tag="ot", name="ot")
                    nc.vector.tensor_copy(ot[:], ops[k][:])
                    nc.sync.dma_start(
                        outT[et * 128:(et + 1) * 128,
                             (NQT - 1) * 512:NQT * 512],
                        ot[:],
                    )
    _split_multi_waits(nc)
    return nc


_PROGRAM = None


def _get_program():
    global _PROGRAM
    if _PROGRAM is None:
        _PROGRAM = build_program()
    return _PROGRAM


def _make_in_maps(query, key, value, Wq, bq, Wk, bk, Wv, bv, Wo):
    bf = ml_dtypes.bfloat16
    # pair-masks for the two diagonal k-chunk pairs of each q-tile:
    # block offsets (0, 128) and (256, 384); keep iff q >= k + o
    k_idx = np.arange(128, dtype=np.int32)[:, None]
    q_idx = np.arange(512, dtype=np.int32)[None, :]
    halves = [(q_idx >= k_idx + o) for o in (0, 128, 256, 384)]
    maskT = np.concatenate(halves, axis=1).astype(bf)
    in_maps = []
    for c in range(8):
        b, g = divmod(c, 2)
        gs = slice(DG * g, DG * (g + 1))
        in_maps.append({
            "xqT": np.asarray(query[b], np.float32).T.astype(bf, order="C"),
            "xkT": np.asarray(key[b], np.float32).T.astype(bf, order="C"),
            "xvT": np.asarray(value[b], np.float32).T.astype(bf, order="C"),
            "wqT": np.asarray(Wq[gs, :], np.float32).T.astype(bf, order="C"),
            "wkT": np.asarray(Wk[gs, :], np.float32).T.astype(bf, order="C"),
            "wvT": np.asarray(Wv[gs, :], np.float32).T.astype(bf, order="C"),
            "woT": np.asarray(Wo[:, gs], np.float32).T.astype(bf, order="C"),
            "bq": np.ascontiguousarray(
                np.asarray(bq[gs], np.float32).reshape(4, 128).T),
            "bk": np.ascontiguousarray(
                np.asarray(bk[gs], np.float32).reshape(4, 128).T),
            "maskT": maskT,
        })
    return in_maps


def run(query, key, value, Wq, bq, Wk, bk, Wv, bv, Wo, bo, trace=False,
        **spmd_kwargs):
    nc = _get_program()
    in_maps = _make_in_maps(query, key, value, Wq, bq, Wk, bk, Wv, bv, Wo)
    res = run_bass_kernel_spmd(nc, in_maps, list(range(8)), trace=trace,
                               **spmd_kwargs)
    out = np.empty((B, S, D), np.float32)
    for b in range(B):
        out[b] = (res.results[2 * b]["outT"] + res.results[2 * b + 1]["outT"]).T
    # bv is folded in host-side: attn rows sum to 1 after softmax, so
    # out += Wo @ bv exactly accounts for the V bias.
    bias = np.asarray(bo, np.float32) + \
        np.asarray(Wo, np.float32) @ np.asarray(bv, np.float32)
    out += bias[None, None, :]
    return out, res


def kernel(query, key, value, mask, Wq, bq, Wk, bk, Wv, bv, Wo, bo):
    out, _ = run(query, key, value, Wq, bq, Wk, bk, Wv, bv, Wo, bo)
    return out
